# revision 13
# baseline (speedup 1.0000x reference)
"""AttentiveStatistiC kernel for 8x TRN2 NeuronCores (Bass/Tile).

Math restructuring (k/v never materialized):
  zs = sum_t h                                   (4096,) per dataset (scale folded)
  qt[(c,sp),n] = sum_c2 A_n[c2,c] zs[c2,sp]      A_n = Wq[n].T @ Wk[n]  (host-fused)
  sim[n,t] = qt_n . h_t                          contraction over feat=4096
  att = softmax_t(sim * scale/T)
  hbarT[feat,n] = sum_t h[t,feat] att[n,t]       (h-chunk stationary matmuls)
  a[head n rows] = Wv[n] @ hbar_n (+bv)
  x = a; 3x ELU(x @ Wl.T + bl); p = x @ Wout.T + bout -> mean/logvar

Sharding: data-parallel over batch for attention (4 datasets/core);
feature-parallel PostPool (512 cols/core) with AllGather between layers.
h is shipped in natural (t-part) and transposed (feat-part) layouts, bf16,
with the feature axis globally reordered sp-major (feat' = sp*256 + c) so
every layout change on chip is either a plain strided slice or a PE
transpose -- no element-granularity DMA scatters.
"""

from contextlib import ExitStack

import numpy as np
import ml_dtypes

import concourse.bass as bass
import concourse.mybir as mybir
import concourse.tile as tile
from concourse import bacc
from concourse.bass_utils import run_bass_kernel_spmd
from concourse.masks import make_identity

BF16 = ml_dtypes.bfloat16
F32 = mybir.dt.float32
F32R = mybir.dt.float32r
BF = mybir.dt.bfloat16
AF = mybir.ActivationFunctionType

B, T, C, NH, DH, SP = 32, 512, 256, 4, 64, 16
FEAT = C * SP            # 4096
CD = 128                 # c_dim
NCORES = 8
BL = B // NCORES         # 4 local datasets per core
CS = FEAT // NCORES      # 512 postpool cols per core
OS = 2 * CD // NCORES    # 64 final cols per core
SCALE = float(1.0 / np.sqrt(np.float32(SP * DH)))
RG = [list(range(NCORES))]

_CACHE = {}


def _r32(ap):
    return ap.bitcast(F32R)


def _build_attention(stk, tc, nc, t_in, t_out, dram, ident):
    hn, ht = t_in["hn"], t_in["ht"]
    att_o, a_o = t_out["att_o"], t_out["a_o"]

    hn_pool = stk.enter_context(tc.tile_pool(name="hn", bufs=8))
    ht_pool = stk.enter_context(tc.tile_pool(name="htp", bufs=40))
    sm = stk.enter_context(tc.tile_pool(name="sm", bufs=3))
    cst = stk.enter_context(tc.tile_pool(name="cst", bufs=1))
    ps = stk.enter_context(tc.tile_pool(name="ps", bufs=6, space="PSUM"))

    # constants: fused qk weights, beta' (=T*beta), WvT, bv
    qkw_sb = cst.tile([128, NH * 2 * C], F32R, name="qkw_sb")      # [n, ch, c]
    beta_sb = cst.tile([128, NH * 2], F32, name="beta_sb")        # [n, m]
    wvt_sb = cst.tile([128, NH * 2 * DH], F32R, name="wvt_sb")     # [n, m, o]
    for n in range(NH):
        for ch in range(2):
            nc.sync.dma_start(qkw_sb[:, (n * 2 + ch) * C:(n * 2 + ch + 1) * C],
                              t_in["qkw"][n, ch])
            nc.sync.dma_start(beta_sb[:, n * 2 + ch:n * 2 + ch + 1],
                              t_in["qkbeta"][n, ch].unsqueeze(-1))
            nc.sync.dma_start(wvt_sb[:, (n * 2 + ch) * DH:(n * 2 + ch + 1) * DH],
                              t_in["wvt"][n, ch])
    bv_sb = cst.tile([DH, NH], F32, name="bv_sb")
    nc.sync.dma_start(bv_sb, t_in["bvh"])

    at_in = dram["at_in"]

    for b in range(BL):
        # ---- load transposed h_b; column sums give zs (feat'-partitioned) ----
        ht_t = []
        z_cs = sm.tile([128, 32], F32R, name=f"z_cs_{b}", tag="z_cs")  # col k'=(sp,ch)
        for k in range(32):
            t_ = ht_pool.tile([128, T], BF, name=f"ht_{b}_{k}", tag="ht")
            nc.sync.dma_start(t_, ht[b, k])
            with nc.allow_low_precision(reason="fp32r accumulate (4-byte f32 storage)"):
                nc.vector.tensor_reduce(z_cs[:, k:k + 1], t_,
                                        axis=mybir.AxisListType.X,
                                        op=mybir.AluOpType.add)
            ht_t.append(t_)
        # natural-layout tiles (used by hbar much later)
        hn_t = [hn_pool.tile([128, FEAT], BF, name=f"hn_{b}_{tt}", tag="hn")
                for tt in range(4)]
        for tt in range(4):
            nc.sync.dma_start(hn_t[tt], hn[b, tt])

        # ---- qt = A_n @ zs per head (+beta'), bf16 ----
        z_csv = z_cs.rearrange("p (sp ch) -> p ch sp", sp=16, ch=2)
        qt_raw = sm.tile([128, NH * 2 * SP], BF, name=f"qtr_{b}", tag="qtr")  # [n, m, sp]
        for n in range(NH):
            for m in range(2):
                q_ps = ps.tile([128, SP], F32, name=f"q_ps_{b}_{n}_{m}", tag="ps")
                for ch in range(2):
                    lhs = qkw_sb[:, (n * 2 + ch) * C + m * 128:(n * 2 + ch) * C + m * 128 + 128]
                    nc.tensor.matmul(q_ps, lhs, z_csv[:, ch],
                                     start=(ch == 0), stop=(ch == 1))
                nc.scalar.activation(qt_raw[:, (n * 2 + m) * SP:(n * 2 + m + 1) * SP], q_ps,
                                     AF.Identity, bias=beta_sb[:, n * 2 + m:n * 2 + m + 1])

        # ---- sim: accumulate over 32 feat' chunks ----
        qtv = qt_raw.rearrange("p (n m sp) -> p m sp n", n=NH, m=2)
        sim_ps = ps.tile([NH, T], F32, name=f"sim_ps_{b}", tag="ps")
        for k in range(32):
            nc.tensor.matmul(sim_ps, qtv[:, k % 2, k // 2], ht_t[k],
                             start=(k == 0), stop=(k == 31))

        # ---- softmax (scale/T folded; zs carried T*z) ----
        sc = SCALE / T
        m4 = sm.tile([NH, 1], F32, name=f"m4_{b}", tag="m4")
        nc.vector.tensor_reduce(m4, sim_ps, axis=mybir.AxisListType.X,
                                op=mybir.AluOpType.max)
        m4n = sm.tile([NH, 1], F32, name=f"m4n_{b}", tag="m4n")
        nc.vector.tensor_scalar_mul(m4n, m4, -sc)
        att_e = sm.tile([NH, T], F32, name=f"att_e_{b}", tag="att_e")
        den = sm.tile([NH, 1], F32, name=f"den_{b}", tag="den")
        nc.scalar.activation(att_e, sim_ps, AF.Exp, bias=m4n, scale=sc, accum_out=den)
        rden = sm.tile([NH, 1], F32, name=f"rden_{b}", tag="rden")
        nc.vector.reciprocal(rden, den)
        att_sb = sm.tile([NH, T], F32, name=f"att_sb_{b}", tag="att_sb")
        nc.vector.tensor_scalar_mul(att_sb, att_e, rden)
        nc.sync.dma_start(att_o[b], att_sb)
        att_bf = sm.tile([NH, T], BF, name=f"att_bf_{b}", tag="att_bf")
        nc.vector.tensor_copy(att_bf, att_sb)

        # ---- attT via PE transpose: (4, 512) -> (t-part, [tt, n]) ----
        attT = sm.tile([128, 16], BF, name=f"attT_{b}", tag="attT")
        for tt in range(4):
            tp_ps = ps.tile([128, NH], BF, name=f"atp_{b}_{tt}", tag="ps")
            nc.tensor.transpose(tp_ps, att_bf[:, tt * 128:(tt + 1) * 128],
                                ident[:NH, :NH])
            nc.vector.tensor_copy(attT[:, tt * NH:(tt + 1) * NH], tp_ps)

        # ---- hbarT directly: h-chunk stationary, out (feat'-part, heads) ----
        hbT = sm.tile([128, 128], F32R, name=f"hbT_{b}", tag="hbT")  # [sp, m, n]
        for k in range(32):
            hb_ps = ps.tile([128, NH], F32, name=f"hb_ps_{b}_{k}", tag="ps")
            for tt in range(4):
                nc.tensor.matmul(hb_ps, hn_t[tt][:, k * 128:(k + 1) * 128],
                                 attT[:, tt * NH:(tt + 1) * NH],
                                 start=(tt == 0), stop=(tt == 3))
            nc.vector.tensor_copy(hbT[:, k * NH:(k + 1) * NH], hb_ps)

        # ---- a = Wv @ hbar per head (+bv) ----
        a_sb = sm.tile([DH, NH * SP], F32, name=f"a_sb_{b}", tag="a_sb")  # [n, sp]
        hbv = hbT.rearrange("p (sp m n) -> p n m sp", sp=16, m=2)
        for n in range(NH):
            a_ps = ps.tile([DH, SP], F32, name=f"a_ps_{b}_{n}", tag="ps")
            for m in range(2):
                lhs = wvt_sb[:, (n * 2 + m) * DH:(n * 2 + m + 1) * DH]
                nc.tensor.matmul(a_ps, lhs, hbv[:, n, m],
                                 start=(m == 0), stop=(m == 1))
            nc.scalar.activation(a_sb[:, n * SP:(n + 1) * SP], a_ps,
                                 AF.Identity, bias=bv_sb[:, n:n + 1])

        # outputs: a_o (f32, c-major) and at_in row (bf16) for the AllGather
        nc.sync.dma_start(a_o[b].rearrange("(n o) sp -> o n sp", n=NH),
                          a_sb.rearrange("o (n sp) -> o n sp", n=NH))
        at_bf = sm.tile([DH, NH * SP], BF, name=f"at_bf_{b}", tag="at_bf")
        nc.vector.tensor_copy(at_bf, a_sb)
        nc.gpsimd.dma_start(at_in[b].rearrange("(n o sp) -> o n sp", n=NH, o=DH),
                            at_bf.rearrange("o (n sp) -> o n sp", n=NH))


def _build_postpool(stk, tc, nc, t_in, t_out, dram, ident):
    p_o = t_out["p_o"]
    w_names = ["w0t", "w1t", "w2t"]
    b_names = ["b0s", "b1s", "b2s"]

    wp = stk.enter_context(tc.tile_pool(name="wp", bufs=8))
    xp = stk.enter_context(tc.tile_pool(name="xp", bufs=34))
    sm2 = stk.enter_context(tc.tile_pool(name="sm2", bufs=3))
    cst2 = stk.enter_context(tc.tile_pool(name="cst2", bufs=1))
    ps2 = stk.enter_context(tc.tile_pool(name="ps2", bufs=4, space="PSUM"))

    ones32 = cst2.tile([1, B], F32R, name="ones32")
    nc.sync.dma_start(ones32, t_in["ones32d"])
    brow = cst2.tile([1, CS * 3 + OS], F32R, name="brow")
    for li in range(3):
        nc.sync.dma_start(brow[:, li * CS:(li + 1) * CS], t_in[b_names[li]])
    nc.sync.dma_start(brow[:, 3 * CS:3 * CS + OS], t_in["bos"])

    # AllGather of attention outputs -> a_all (32, 4096) batch-major bf16
    nc.gpsimd.collective_compute("AllGather", mybir.AluOpType.bypass,
                                 ins=[dram["at_in"].opt()],
                                 outs=[dram["at_all"].opt()],
                                 replica_groups=RG)

    # transpose a_all once into feature-major lhsT tiles
    aal = sm2.tile([B, FEAT], BF, name="aal", tag="aal")
    nc.sync.dma_start(aal, dram["at_all"])
    xt_tiles = []
    for kk in range(32):
        xtp = ps2.tile([128, B], BF, name=f"xtp_{kk}", tag="ps2")
        nc.tensor.transpose(xtp, aal[:, kk * 128:(kk + 1) * 128], ident[:B, :B])
        xt = xp.tile([128, B], BF, name=f"xt0_{kk}", tag="xt")
        nc.vector.tensor_copy(xt, xtp)
        xt_tiles.append(xt)

    for li in range(4):
        last = li == 3
        ncols = OS if last else CS
        x_ps = ps2.tile([B, ncols], F32, name=f"x_ps_{li}", tag="ps2")
        wt = t_in["wot"] if last else t_in[w_names[li]]
        for kk in range(32):
            w_t = wp.tile([128, ncols], BF, name=f"w_{li}_{kk}", tag="w")
            nc.sync.dma_start(w_t, wt[kk])
            nc.tensor.matmul(x_ps, xt_tiles[kk], w_t, start=(kk == 0), stop=False)
        # bias via K=1 ones-column matmul folded into the accumulation group
        boff = 3 * CS if last else li * CS
        nc.tensor.matmul(x_ps, ones32, brow[:, boff:boff + ncols],
                         start=False, stop=True)
        if last:
            p_sb = sm2.tile([B, OS], F32, name="p_sb", tag="p_sb")
            nc.vector.tensor_copy(p_sb, x_ps)
            nc.sync.dma_start(p_o, p_sb)
            break

        # ELU(v) = relu(v) + exp(min(v, 0)) - 1
        t_sb = sm2.tile([B, CS], F32, name=f"t_{li}", tag="t")
        nc.vector.tensor_scalar_min(t_sb, x_ps, 0.0)
        e_sb = sm2.tile([B, CS], F32, name=f"e_{li}", tag="e")
        nc.scalar.activation(e_sb, t_sb, AF.Exp)
        r_sb = sm2.tile([B, CS], F32, name=f"r_{li}", tag="r")
        nc.scalar.activation(r_sb, x_ps, AF.Relu)
        s_sb = sm2.tile([B, CS], F32, name=f"s_{li}", tag="s")
        nc.vector.tensor_tensor(out=s_sb, in0=r_sb, in1=e_sb, op=mybir.AluOpType.add)
        y_bf = sm2.tile([B, CS], BF, name=f"y_{li}", tag="y")
        nc.vector.tensor_scalar_add(y_bf, s_sb, -1.0)

        # transpose y (32, 512) -> x_in (512, 32) via PE, then AllGather
        x_in = dram["x_in"][li]
        xt_tiles = []
        for cc in range(4):
            ytp = ps2.tile([128, B], BF, name=f"ytp_{li}_{cc}", tag="ps2")
            nc.tensor.transpose(ytp, y_bf[:, cc * 128:(cc + 1) * 128], ident[:B, :B])
            yt = sm2.tile([128, B], BF, name=f"yt_{li}_{cc}", tag="yt")
            nc.vector.tensor_copy(yt, ytp)
            nc.sync.dma_start(x_in[cc * 128:(cc + 1) * 128], yt)
        x_all = dram["x_all"][li]
        nc.gpsimd.collective_compute("AllGather", mybir.AluOpType.bypass,
                                     ins=[x_in.opt()], outs=[x_all.opt()],
                                     replica_groups=RG)
        for kk in range(32):
            xt = xp.tile([128, B], BF, name=f"xt{li + 1}_{kk}", tag="xt")
            nc.sync.dma_start(xt, x_all[kk * 128:(kk + 1) * 128])
            xt_tiles.append(xt)


def _build_nc():
    nc = bacc.Bacc("TRN2", target_bir_lowering=False, debug=False,
                   enable_asserts=False, num_devices=NCORES)
    t_in = {
        "hn": nc.dram_tensor("hn", [BL, 4, 128, FEAT], BF, kind="ExternalInput").ap(),
        "ht": nc.dram_tensor("ht", [BL, 32, 128, T], BF, kind="ExternalInput").ap(),
        "qkw": nc.dram_tensor("qkw", [NH, 2, 128, C], F32R, kind="ExternalInput").ap(),
        "qkbeta": nc.dram_tensor("qkbeta", [NH, 2, 128], F32, kind="ExternalInput").ap(),
        "wvt": nc.dram_tensor("wvt", [NH, 2, 128, DH], F32R, kind="ExternalInput").ap(),
        "bvh": nc.dram_tensor("bvh", [DH, NH], F32, kind="ExternalInput").ap(),
        "w0t": nc.dram_tensor("w0t", [32, 128, CS], BF, kind="ExternalInput").ap(),
        "w1t": nc.dram_tensor("w1t", [32, 128, CS], BF, kind="ExternalInput").ap(),
        "w2t": nc.dram_tensor("w2t", [32, 128, CS], BF, kind="ExternalInput").ap(),
        "wot": nc.dram_tensor("wot", [32, 128, OS], BF, kind="ExternalInput").ap(),
        "b0s": nc.dram_tensor("b0s", [1, CS], F32R, kind="ExternalInput").ap(),
        "b1s": nc.dram_tensor("b1s", [1, CS], F32R, kind="ExternalInput").ap(),
        "b2s": nc.dram_tensor("b2s", [1, CS], F32R, kind="ExternalInput").ap(),
        "bos": nc.dram_tensor("bos", [1, OS], F32R, kind="ExternalInput").ap(),
        "ones32d": nc.dram_tensor("ones32d", [1, B], F32R, kind="ExternalInput").ap(),
    }
    t_out = {
        "att_o": nc.dram_tensor("att_o", [BL, NH, T], F32, kind="ExternalOutput").ap(),
        "a_o": nc.dram_tensor("a_o", [BL, C, SP], F32, kind="ExternalOutput").ap(),
        "p_o": nc.dram_tensor("p_o", [B, OS], F32, kind="ExternalOutput").ap(),
    }

    with tile.TileContext(nc) as tc:
        with tc.tile_pool(name="dram", bufs=1, space="DRAM") as dpool, \
             tc.tile_pool(name="identp", bufs=1) as identp:
            ident = identp.tile([128, 128], BF, name="ident")
            make_identity(nc, ident)
            dram = {
                "at_in": dpool.tile([BL, FEAT], BF, name="at_in"),
                "at_all": dpool.tile([B, FEAT], BF, name="at_all",
                                     addr_space="Shared"),
                "x_in": [dpool.tile([CS, B], BF, name=f"x_in{i}") for i in range(3)],
                "x_all": [dpool.tile([FEAT, B], BF, name=f"x_all{i}",
                                     addr_space="Shared") for i in range(3)],
            }
            with ExitStack() as stk1:
                _build_attention(stk1, tc, nc, t_in, t_out, dram, ident)
            with ExitStack() as stk2:
                _build_postpool(stk2, tc, nc, t_in, t_out, dram, ident)

    nc.compile()
    return nc


def _prep_inputs(h, Wq, bq, Wk, bk, Wv, bv, W0, b0, W1, b1, W2, b2, Wout, bout):
    """Host-side slicing/transposition per core. Returns in_maps list."""
    f32 = np.float32
    h2 = np.ascontiguousarray(np.asarray(h, f32)).reshape(B, T, C, SP)
    assert not np.any(np.asarray(bk)), "nonzero bk not supported by this kernel build"

    # fused per-head weights; beta scaled by T (zs carries T*z)
    A = np.stack([np.asarray(Wq)[n * DH:(n + 1) * DH, :].T @
                  np.asarray(Wk)[n * DH:(n + 1) * DH, :] for n in range(NH)])
    beta = np.stack([np.asarray(Wk)[n * DH:(n + 1) * DH, :].T @
                     np.asarray(bq)[n * DH:(n + 1) * DH] for n in range(NH)]) * T
    qkw = np.ascontiguousarray(A.reshape(NH, 2, 128, C), f32)
    qkbeta = np.ascontiguousarray(beta.reshape(NH, 2, 128), f32)
    wvt = np.ascontiguousarray(
        np.stack([np.asarray(Wv)[n * DH:(n + 1) * DH, :].T.reshape(2, 128, DH)
                  for n in range(NH)]), f32)
    bvh = np.ascontiguousarray(np.asarray(bv, f32).reshape(NH, DH).T)

    Ws = [np.asarray(W0), np.asarray(W1), np.asarray(W2)]
    bs_ = [np.asarray(b0, f32), np.asarray(b1, f32), np.asarray(b2, f32)]
    in_maps = []
    for i in range(NCORES):
        sl = slice(i * BL, (i + 1) * BL)
        h4 = h2[sl]
        # feat' = sp*256 + c (sp-major reorder; contraction order-invariant)
        hn_i = np.ascontiguousarray(h4.transpose(0, 1, 3, 2)).reshape(
            BL, 4, 128, FEAT).astype(BF16)
        ht_i = np.ascontiguousarray(h4.transpose(0, 3, 2, 1)).reshape(
            BL, 32, 128, T).astype(BF16)
        cols = slice(i * CS, (i + 1) * CS)
        m = {
            "hn": np.ascontiguousarray(hn_i),
            "ht": np.ascontiguousarray(ht_i),
            "qkw": qkw, "qkbeta": qkbeta, "wvt": wvt, "bvh": bvh,
            "b0s": np.ascontiguousarray(bs_[0][None, cols]),
            "b1s": np.ascontiguousarray(bs_[1][None, cols]),
            "b2s": np.ascontiguousarray(bs_[2][None, cols]),
            "bos": np.ascontiguousarray(np.asarray(bout, f32)[None, i * OS:(i + 1) * OS]),
            "ones32d": np.ones((1, B), f32),
            "wot": np.ascontiguousarray(
                np.asarray(Wout)[i * OS:(i + 1) * OS, :].T.reshape(32, 128, OS)
            ).astype(BF16),
        }
        for li, W in enumerate(Ws):
            m[f"w{li}t"] = np.ascontiguousarray(
                W[cols, :].T.reshape(32, 128, CS)).astype(BF16)
        in_maps.append(m)
    return in_maps


def kernel(h, Wq, bq, Wk, bk, Wv, bv, W0, b0, W1, b1, W2, b2, Wout, bout,
           bs, ns, **kw):
    assert int(bs) == B and int(ns) == T
    if "nc" not in _CACHE:
        _CACHE["nc"] = _build_nc()
    nc = _CACHE["nc"]
    in_maps = _prep_inputs(h, Wq, bq, Wk, bk, Wv, bv, W0, b0, W1, b1, W2, b2,
                           Wout, bout)
    res = run_bass_kernel_spmd(nc, in_maps, core_ids=list(range(NCORES)),
                               trace=bool(kw.get("trace", False)))
    _CACHE["last_results"] = res
    outs = res.results
    att = np.concatenate([o["att_o"] for o in outs], axis=0)          # (B, NH, T)
    a = np.concatenate([o["a_o"] for o in outs], axis=0)              # (B, C, SP)
    p = np.concatenate([o["p_o"] for o in outs], axis=1)              # (B, 512)
    mean, logvar = p[:, :CD], p[:, CD:]
    return (np.ascontiguousarray(mean, np.float32),
            np.ascontiguousarray(logvar, np.float32),
            np.ascontiguousarray(a.reshape(B, C, 4, 4), np.float32),
            np.ascontiguousarray(att.reshape(B, NH, 1, T), np.float32))


# revision 14
# speedup vs baseline: 1.2817x; 1.2817x over previous
"""AttentiveStatistiC kernel for 8x TRN2 NeuronCores (Bass/Tile).

Math restructuring (k/v never materialized):
  zs = sum_t h                                   (4096,) per dataset (scale folded)
  qt[(c,sp),n] = sum_c2 A_n[c2,c] zs[c2,sp]      A_n = Wq[n].T @ Wk[n]  (host-fused)
  sim[n,t] = qt_n . h_t                          contraction over feat=4096
  att = softmax_t(sim * scale/T)
  hbarT[feat,n] = sum_t h[t,feat] att[n,t]       (h-chunk stationary matmuls)
  a[head n rows] = Wv[n] @ hbar_n (+bv)
  x = a; 3x ELU(x @ Wl.T + bl); p = x @ Wout.T + bout -> mean/logvar

Sharding: data-parallel over batch for attention (4 datasets/core);
feature-parallel PostPool (512 cols/core) with AllGather between layers.
h is shipped in natural (t-part) and transposed (feat-part) layouts, bf16,
with the feature axis globally reordered sp-major (feat' = sp*256 + c) so
every layout change on chip is either a plain strided slice or a PE
transpose -- no element-granularity DMA scatters.
"""

from contextlib import ExitStack

import numpy as np
import ml_dtypes

import concourse.bass as bass
import concourse.mybir as mybir
import concourse.tile as tile
from concourse import bacc
from concourse.bass_utils import run_bass_kernel_spmd
from concourse.masks import make_identity

BF16 = ml_dtypes.bfloat16
F32 = mybir.dt.float32
F32R = mybir.dt.float32r
BF = mybir.dt.bfloat16
AF = mybir.ActivationFunctionType

B, T, C, NH, DH, SP = 32, 512, 256, 4, 64, 16
FEAT = C * SP            # 4096
CD = 128                 # c_dim
NCORES = 8
BL = B // NCORES         # 4 local datasets per core
CS = FEAT // NCORES      # 512 postpool cols per core
OS = 2 * CD // NCORES    # 64 final cols per core
SCALE = float(1.0 / np.sqrt(np.float32(SP * DH)))
RG = [list(range(NCORES))]

_CACHE = {}


def _r32(ap):
    return ap.bitcast(F32R)


def _build_attention(stk, tc, nc, t_in, t_out, dram, ident):
    hn, ht = t_in["hn"], t_in["ht"]
    att_o, a_o = t_out["att_o"], t_out["a_o"]

    hn_pool = stk.enter_context(tc.tile_pool(name="hn", bufs=6))
    ht_pool = stk.enter_context(tc.tile_pool(name="htp", bufs=12))
    sm = stk.enter_context(tc.tile_pool(name="sm", bufs=3))
    cst = stk.enter_context(tc.tile_pool(name="cst", bufs=1))
    ps = stk.enter_context(tc.tile_pool(name="ps", bufs=6, space="PSUM"))

    # constants: fused qk weights, beta' (=T*beta), WvT, bv
    qkw_sb = cst.tile([128, NH * 2 * C], BF, name="qkw_sb")      # [n, ch, c]
    beta_sb = cst.tile([128, NH * 2], F32, name="beta_sb")        # [n, m]
    wvt_sb = cst.tile([128, NH * 2 * DH], BF, name="wvt_sb")     # [n, m, o]
    for n in range(NH):
        for ch in range(2):
            nc.sync.dma_start(qkw_sb[:, (n * 2 + ch) * C:(n * 2 + ch + 1) * C],
                              t_in["qkw"][n, ch])
            nc.sync.dma_start(beta_sb[:, n * 2 + ch:n * 2 + ch + 1],
                              t_in["qkbeta"][n, ch].unsqueeze(-1))
            nc.sync.dma_start(wvt_sb[:, (n * 2 + ch) * DH:(n * 2 + ch + 1) * DH],
                              t_in["wvt"][n, ch])
    bv_sb = cst.tile([DH, NH], F32, name="bv_sb")
    nc.sync.dma_start(bv_sb, t_in["bvh"])

    at_in = dram["at_in"]

    for b in range(BL):
        # ---- load transposed h_b (4 chunks per DMA); column sums -> zs ----
        ht_t = []
        z_cs = sm.tile([128, 32], BF, name=f"z_cs_{b}", tag="z_cs")  # col k'=(sp,ch)
        for g in range(8):
            t_ = ht_pool.tile([128, 4 * T], BF, name=f"ht_{b}_{g}", tag="ht")
            nc.sync.dma_start(t_.rearrange("p (k t) -> p k t", k=4),
                              ht[b, 4 * g:4 * (g + 1)].rearrange("k p t -> p k t"))
            with nc.allow_low_precision(reason="bf16 z accumulate, validated numerically"):
                nc.vector.tensor_reduce(z_cs[:, 4 * g:4 * (g + 1)],
                                        t_.rearrange("p (k t) -> p k t", k=4),
                                        axis=mybir.AxisListType.X,
                                        op=mybir.AluOpType.add)
            ht_t.append(t_)
        # natural-layout tiles (used by hbar much later)
        hn_t = [hn_pool.tile([128, FEAT], BF, name=f"hn_{b}_{tt}", tag="hn")
                for tt in range(4)]
        for tt in range(4):
            nc.scalar.dma_start(hn_t[tt], hn[b, tt])

        # ---- qt = A_n @ zs per head (+beta'), bf16 ----
        z_csv = z_cs.rearrange("p (sp ch) -> p ch sp", sp=16, ch=2)
        qt_raw = sm.tile([128, NH * 2 * SP], BF, name=f"qtr_{b}", tag="qtr")  # [n, m, sp]
        for n in range(NH):
            for m in range(2):
                q_ps = ps.tile([128, SP], F32, name=f"q_ps_{b}_{n}_{m}", tag="ps")
                for ch in range(2):
                    lhs = qkw_sb[:, (n * 2 + ch) * C + m * 128:(n * 2 + ch) * C + m * 128 + 128]
                    nc.tensor.matmul(q_ps, lhs, z_csv[:, ch],
                                     start=(ch == 0), stop=(ch == 1))
                nc.scalar.activation(qt_raw[:, (n * 2 + m) * SP:(n * 2 + m + 1) * SP], q_ps,
                                     AF.Identity, bias=beta_sb[:, n * 2 + m:n * 2 + m + 1])

        # ---- sim: accumulate over 32 feat' chunks ----
        qtv = qt_raw.rearrange("p (n m sp) -> p m sp n", n=NH, m=2)
        sim_ps = ps.tile([NH, T], F32, name=f"sim_ps_{b}", tag="ps")
        for k in range(32):
            nc.tensor.matmul(sim_ps, qtv[:, k % 2, k // 2],
                             ht_t[k // 4][:, (k % 4) * T:(k % 4 + 1) * T],
                             start=(k == 0), stop=(k == 31))

        # ---- softmax (scale/T folded; zs carried T*z) ----
        sc = SCALE / T
        m4 = sm.tile([NH, 1], F32, name=f"m4_{b}", tag="m4")
        nc.vector.tensor_reduce(m4, sim_ps, axis=mybir.AxisListType.X,
                                op=mybir.AluOpType.max)
        m4n = sm.tile([NH, 1], F32, name=f"m4n_{b}", tag="m4n")
        nc.vector.tensor_scalar_mul(m4n, m4, -sc)
        att_e = sm.tile([NH, T], F32, name=f"att_e_{b}", tag="att_e")
        den = sm.tile([NH, 1], F32, name=f"den_{b}", tag="den")
        nc.scalar.activation(att_e, sim_ps, AF.Exp, bias=m4n, scale=sc, accum_out=den)
        rden = sm.tile([NH, 1], F32, name=f"rden_{b}", tag="rden")
        nc.vector.reciprocal(rden, den)
        att_sb = sm.tile([NH, T], F32, name=f"att_sb_{b}", tag="att_sb")
        nc.vector.tensor_scalar_mul(att_sb, att_e, rden)
        nc.sync.dma_start(att_o[b], att_sb)
        att_bf = sm.tile([NH, T], BF, name=f"att_bf_{b}", tag="att_bf")
        nc.vector.tensor_copy(att_bf, att_sb)

        # ---- attT via PE transpose: (4, 512) -> (t-part, [tt, n]) ----
        attT = sm.tile([128, 16], BF, name=f"attT_{b}", tag="attT")
        for tt in range(4):
            tp_ps = ps.tile([128, NH], BF, name=f"atp_{b}_{tt}", tag="ps")
            nc.tensor.transpose(tp_ps, att_bf[:, tt * 128:(tt + 1) * 128],
                                ident[:NH, :NH])
            nc.vector.tensor_copy(attT[:, tt * NH:(tt + 1) * NH], tp_ps)

        # ---- hbarT directly: h-chunk stationary, out (feat'-part, heads) ----
        hbT = sm.tile([128, 128], BF, name=f"hbT_{b}", tag="hbT")  # [sp, m, n]
        for k in range(32):
            hb_ps = ps.tile([128, NH], F32, name=f"hb_ps_{b}_{k}", tag="ps")
            for tt in range(4):
                nc.tensor.matmul(hb_ps, hn_t[tt][:, k * 128:(k + 1) * 128],
                                 attT[:, tt * NH:(tt + 1) * NH],
                                 start=(tt == 0), stop=(tt == 3))
            with nc.allow_low_precision(reason="hbar to bf16, validated numerically"):
                nc.vector.tensor_copy(hbT[:, k * NH:(k + 1) * NH], hb_ps)

        # ---- a = Wv @ hbar per head (+bv) ----
        a_sb = sm.tile([DH, NH * SP], F32, name=f"a_sb_{b}", tag="a_sb")  # [n, sp]
        hbv = hbT.rearrange("p (sp m n) -> p n m sp", sp=16, m=2)
        for n in range(NH):
            a_ps = ps.tile([DH, SP], F32, name=f"a_ps_{b}_{n}", tag="ps")
            for m in range(2):
                lhs = wvt_sb[:, (n * 2 + m) * DH:(n * 2 + m + 1) * DH]
                nc.tensor.matmul(a_ps, lhs, hbv[:, n, m],
                                 start=(m == 0), stop=(m == 1))
            nc.scalar.activation(a_sb[:, n * SP:(n + 1) * SP], a_ps,
                                 AF.Identity, bias=bv_sb[:, n:n + 1])

        # outputs: a_o (f32, c-major) and at_in row (bf16) for the AllGather
        nc.sync.dma_start(a_o[b].rearrange("(n o) sp -> o n sp", n=NH),
                          a_sb.rearrange("o (n sp) -> o n sp", n=NH))
        at_bf = sm.tile([DH, NH * SP], BF, name=f"at_bf_{b}", tag="at_bf")
        nc.vector.tensor_copy(at_bf, a_sb)
        nc.gpsimd.dma_start(at_in[b].rearrange("(n o sp) -> o n sp", n=NH, o=DH),
                            at_bf.rearrange("o (n sp) -> o n sp", n=NH))


def _build_postpool(stk, tc, nc, t_in, t_out, dram, ident):
    p_o = t_out["p_o"]
    w_names = ["w0t", "w1t", "w2t"]
    b_names = ["b0s", "b1s", "b2s"]

    wp = stk.enter_context(tc.tile_pool(name="wp", bufs=10))
    xp = stk.enter_context(tc.tile_pool(name="xp", bufs=4))
    sm2 = stk.enter_context(tc.tile_pool(name="sm2", bufs=3))
    cst2 = stk.enter_context(tc.tile_pool(name="cst2", bufs=1))
    ps2 = stk.enter_context(tc.tile_pool(name="ps2", bufs=4, space="PSUM"))

    ones32 = cst2.tile([1, B], BF, name="ones32")
    nc.scalar.dma_start(ones32, t_in["ones32d"])
    brow = cst2.tile([1, CS * 3 + OS], BF, name="brow")
    for li in range(3):
        nc.scalar.dma_start(brow[:, li * CS:(li + 1) * CS], t_in[b_names[li]])
    nc.scalar.dma_start(brow[:, 3 * CS:3 * CS + OS], t_in["bos"])

    # AllGather of attention outputs -> a_all (32, 4096) batch-major bf16
    nc.gpsimd.collective_compute("AllGather", mybir.AluOpType.bypass,
                                 ins=[dram["at_in"].opt()],
                                 outs=[dram["at_all"].opt()],
                                 replica_groups=RG)

    # transpose a_all once into a feature-major lhsT tile
    aal = sm2.tile([B, FEAT], BF, name="aal", tag="aal")
    nc.scalar.dma_start(aal, dram["at_all"])
    xt_all = xp.tile([128, 32 * B], BF, name="xt_a0", tag="xt")  # [kk, b]
    for kk in range(32):
        xtp = ps2.tile([128, B], BF, name=f"xtp_{kk}", tag="ps2")
        nc.tensor.transpose(xtp, aal[:, kk * 128:(kk + 1) * 128], ident[:B, :B])
        nc.vector.tensor_copy(xt_all[:, kk * B:(kk + 1) * B], xtp)

    for li in range(4):
        last = li == 3
        ncols = OS if last else CS
        x_ps = ps2.tile([B, ncols], F32, name=f"x_ps_{li}", tag="ps2")
        wt = t_in["wot"] if last else t_in[w_names[li]]
        for g in range(8):
            w_t = wp.tile([128, 4 * CS], BF, name=f"w_{li}_{g}", tag="w")
            nc.sync.dma_start(
                w_t[:, :4 * ncols].rearrange("p (k c) -> p k c", k=4),
                wt[4 * g:4 * (g + 1)].rearrange("k p c -> p k c"))
            for j in range(4):
                kk = 4 * g + j
                nc.tensor.matmul(x_ps, xt_all[:, kk * B:(kk + 1) * B],
                                 w_t[:, j * ncols:(j + 1) * ncols],
                                 start=(kk == 0), stop=False)
        # bias via K=1 ones-column matmul folded into the accumulation group
        boff = 3 * CS if last else li * CS
        nc.tensor.matmul(x_ps, ones32, brow[:, boff:boff + ncols],
                         start=False, stop=True)
        if last:
            p_sb = sm2.tile([B, OS], F32, name="p_sb", tag="p_sb")
            nc.vector.tensor_copy(p_sb, x_ps)
            nc.sync.dma_start(p_o, p_sb)
            break

        # ELU(v) = relu(v) + exp(min(v, 0)) - 1
        t_sb = sm2.tile([B, CS], F32, name=f"t_{li}", tag="t")
        nc.vector.tensor_scalar_min(t_sb, x_ps, 0.0)
        e_sb = sm2.tile([B, CS], F32, name=f"e_{li}", tag="e")
        nc.scalar.activation(e_sb, t_sb, AF.Exp)
        r_sb = sm2.tile([B, CS], F32, name=f"r_{li}", tag="r")
        nc.scalar.activation(r_sb, x_ps, AF.Relu)
        s_sb = sm2.tile([B, CS], F32, name=f"s_{li}", tag="s")
        nc.vector.tensor_tensor(out=s_sb, in0=r_sb, in1=e_sb, op=mybir.AluOpType.add)
        y_bf = sm2.tile([B, CS], BF, name=f"y_{li}", tag="y")
        nc.vector.tensor_scalar_add(y_bf, s_sb, -1.0)

        # transpose y (32, 512) -> x_in (512, 32) via PE, then AllGather
        x_in = dram["x_in"][li]
        for cc in range(4):
            ytp = ps2.tile([128, B], BF, name=f"ytp_{li}_{cc}", tag="ps2")
            nc.tensor.transpose(ytp, y_bf[:, cc * 128:(cc + 1) * 128], ident[:B, :B])
            yt = sm2.tile([128, B], BF, name=f"yt_{li}_{cc}", tag="yt")
            nc.vector.tensor_copy(yt, ytp)
            nc.sync.dma_start(x_in[cc * 128:(cc + 1) * 128], yt)
        x_all = dram["x_all"][li]
        nc.gpsimd.collective_compute("AllGather", mybir.AluOpType.bypass,
                                     ins=[x_in.opt()], outs=[x_all.opt()],
                                     replica_groups=RG)
        xt_all = xp.tile([128, 32 * B], BF, name=f"xt_a{li + 1}", tag="xt")
        nc.scalar.dma_start(
            xt_all.rearrange("p (kk b) -> p kk b", kk=32),
            x_all.rearrange("(kk p) b -> p kk b", kk=32))


def _build_nc():
    nc = bacc.Bacc("TRN2", target_bir_lowering=False, debug=False,
                   enable_asserts=False, num_devices=NCORES)
    t_in = {
        "hn": nc.dram_tensor("hn", [BL, 4, 128, FEAT], BF, kind="ExternalInput").ap(),
        "ht": nc.dram_tensor("ht", [BL, 32, 128, T], BF, kind="ExternalInput").ap(),
        "qkw": nc.dram_tensor("qkw", [NH, 2, 128, C], BF, kind="ExternalInput").ap(),
        "qkbeta": nc.dram_tensor("qkbeta", [NH, 2, 128], F32, kind="ExternalInput").ap(),
        "wvt": nc.dram_tensor("wvt", [NH, 2, 128, DH], BF, kind="ExternalInput").ap(),
        "bvh": nc.dram_tensor("bvh", [DH, NH], F32, kind="ExternalInput").ap(),
        "w0t": nc.dram_tensor("w0t", [32, 128, CS], BF, kind="ExternalInput").ap(),
        "w1t": nc.dram_tensor("w1t", [32, 128, CS], BF, kind="ExternalInput").ap(),
        "w2t": nc.dram_tensor("w2t", [32, 128, CS], BF, kind="ExternalInput").ap(),
        "wot": nc.dram_tensor("wot", [32, 128, OS], BF, kind="ExternalInput").ap(),
        "b0s": nc.dram_tensor("b0s", [1, CS], BF, kind="ExternalInput").ap(),
        "b1s": nc.dram_tensor("b1s", [1, CS], BF, kind="ExternalInput").ap(),
        "b2s": nc.dram_tensor("b2s", [1, CS], BF, kind="ExternalInput").ap(),
        "bos": nc.dram_tensor("bos", [1, OS], BF, kind="ExternalInput").ap(),
        "ones32d": nc.dram_tensor("ones32d", [1, B], BF, kind="ExternalInput").ap(),
    }
    t_out = {
        "att_o": nc.dram_tensor("att_o", [BL, NH, T], F32, kind="ExternalOutput").ap(),
        "a_o": nc.dram_tensor("a_o", [BL, C, SP], F32, kind="ExternalOutput").ap(),
        "p_o": nc.dram_tensor("p_o", [B, OS], F32, kind="ExternalOutput").ap(),
    }

    with tile.TileContext(nc) as tc:
        with tc.tile_pool(name="dram", bufs=1, space="DRAM") as dpool, \
             tc.tile_pool(name="identp", bufs=1) as identp:
            ident = identp.tile([128, 128], BF, name="ident")
            make_identity(nc, ident)
            dram = {
                "at_in": dpool.tile([BL, FEAT], BF, name="at_in"),
                "at_all": dpool.tile([B, FEAT], BF, name="at_all",
                                     addr_space="Shared"),
                "x_in": [dpool.tile([CS, B], BF, name=f"x_in{i}") for i in range(3)],
                "x_all": [dpool.tile([FEAT, B], BF, name=f"x_all{i}",
                                     addr_space="Shared") for i in range(3)],
            }
            with ExitStack() as stk1:
                _build_attention(stk1, tc, nc, t_in, t_out, dram, ident)
            with ExitStack() as stk2:
                _build_postpool(stk2, tc, nc, t_in, t_out, dram, ident)

    nc.compile()
    return nc


def _prep_inputs(h, Wq, bq, Wk, bk, Wv, bv, W0, b0, W1, b1, W2, b2, Wout, bout):
    """Host-side slicing/transposition per core. Returns in_maps list."""
    f32 = np.float32
    h2 = np.ascontiguousarray(np.asarray(h, f32)).reshape(B, T, C, SP)
    assert not np.any(np.asarray(bk)), "nonzero bk not supported by this kernel build"

    # fused per-head weights; beta scaled by T (zs carries T*z)
    A = np.stack([np.asarray(Wq)[n * DH:(n + 1) * DH, :].T @
                  np.asarray(Wk)[n * DH:(n + 1) * DH, :] for n in range(NH)])
    beta = np.stack([np.asarray(Wk)[n * DH:(n + 1) * DH, :].T @
                     np.asarray(bq)[n * DH:(n + 1) * DH] for n in range(NH)]) * T
    qkw = np.ascontiguousarray(A.reshape(NH, 2, 128, C)).astype(BF16)
    qkbeta = np.ascontiguousarray(beta.reshape(NH, 2, 128), f32)
    wvt = np.ascontiguousarray(
        np.stack([np.asarray(Wv)[n * DH:(n + 1) * DH, :].T.reshape(2, 128, DH)
                  for n in range(NH)])).astype(BF16)
    bvh = np.ascontiguousarray(np.asarray(bv, f32).reshape(NH, DH).T)

    Ws = [np.asarray(W0), np.asarray(W1), np.asarray(W2)]
    bs_ = [np.asarray(b0, f32), np.asarray(b1, f32), np.asarray(b2, f32)]
    in_maps = []
    for i in range(NCORES):
        sl = slice(i * BL, (i + 1) * BL)
        h4 = h2[sl]
        # feat' = sp*256 + c (sp-major reorder; contraction order-invariant)
        hn_i = np.ascontiguousarray(h4.transpose(0, 1, 3, 2)).reshape(
            BL, 4, 128, FEAT).astype(BF16)
        ht_i = np.ascontiguousarray(h4.transpose(0, 3, 2, 1)).reshape(
            BL, 32, 128, T).astype(BF16)
        cols = slice(i * CS, (i + 1) * CS)
        m = {
            "hn": np.ascontiguousarray(hn_i),
            "ht": np.ascontiguousarray(ht_i),
            "qkw": qkw, "qkbeta": qkbeta, "wvt": wvt, "bvh": bvh,
            "b0s": np.ascontiguousarray(bs_[0][None, cols]).astype(BF16),
            "b1s": np.ascontiguousarray(bs_[1][None, cols]).astype(BF16),
            "b2s": np.ascontiguousarray(bs_[2][None, cols]).astype(BF16),
            "bos": np.ascontiguousarray(np.asarray(bout, f32)[None, i * OS:(i + 1) * OS]).astype(BF16),
            "ones32d": np.ones((1, B), BF16),
            "wot": np.ascontiguousarray(
                np.asarray(Wout)[i * OS:(i + 1) * OS, :].T.reshape(32, 128, OS)
            ).astype(BF16),
        }
        for li, W in enumerate(Ws):
            m[f"w{li}t"] = np.ascontiguousarray(
                W[cols, :].T.reshape(32, 128, CS)).astype(BF16)
        in_maps.append(m)
    return in_maps


def kernel(h, Wq, bq, Wk, bk, Wv, bv, W0, b0, W1, b1, W2, b2, Wout, bout,
           bs, ns, **kw):
    assert int(bs) == B and int(ns) == T
    if "nc" not in _CACHE:
        _CACHE["nc"] = _build_nc()
    nc = _CACHE["nc"]
    in_maps = _prep_inputs(h, Wq, bq, Wk, bk, Wv, bv, W0, b0, W1, b1, W2, b2,
                           Wout, bout)
    res = run_bass_kernel_spmd(nc, in_maps, core_ids=list(range(NCORES)),
                               trace=bool(kw.get("trace", False)))
    _CACHE["last_results"] = res
    outs = res.results
    att = np.concatenate([o["att_o"] for o in outs], axis=0)          # (B, NH, T)
    a = np.concatenate([o["a_o"] for o in outs], axis=0)              # (B, C, SP)
    p = np.concatenate([o["p_o"] for o in outs], axis=1)              # (B, 512)
    mean, logvar = p[:, :CD], p[:, CD:]
    return (np.ascontiguousarray(mean, np.float32),
            np.ascontiguousarray(logvar, np.float32),
            np.ascontiguousarray(a.reshape(B, C, 4, 4), np.float32),
            np.ascontiguousarray(att.reshape(B, NH, 1, T), np.float32))
